# revision 38
# baseline (speedup 1.0000x reference)
"""Multi-head causal attention (B=4, T=2048, D=1024, 16 heads) on 8 TRN2 cores.

Sharding: core c -> batch b = c//2, head-group g = c%2 (8 of 16 heads).
Each core computes its batch's QKV for its heads, flash-style causal
attention with scores kept transposed (S^T[k, q]) so softmax sums come
free via a ones-column appended to V, then a partial output projection
y_part = attn_local @ W_proj[rows]. Host sums the two head-group partials
per batch.

Schedule: a single fused loop over 512-wide t-chunks. Chunk ntc's QKV
matmuls run, then flash attention for q-chunk ntc (which needs K/V
chunks 0..ntc only, all available). The next chunk's QKV matmuls and the
previous chunk's output-projection matmuls are spliced into the
exp-latency gaps of the flash loop so the PE never idles waiting on the
scalar engine. Softmax normalization is deferred: per head the
unnormalized attention rows and the l-row are evacuated to SBUF
immediately (releasing PSUM), then one batched reciprocal per q-chunk +
a DRAM-bounce partition-broadcast produce 1/l, and the normalize
multiplies run on the vector engine underneath the next chunk's flash.

Matmul operands are fp16; accumulation stays fp32 in PSUM.
"""

import math
from collections import deque
from contextlib import ExitStack

import numpy as np

import concourse.bacc as bacc
import concourse.bass as bass
import concourse.mybir as mybir
import concourse.tile as tile
from concourse.bass_utils import run_bass_kernel_spmd

AF = mybir.ActivationFunctionType
F32 = mybir.dt.float32
F16 = mybir.dt.float16

B_FULL = 4
T_FULL = 2048
D_FULL = 1024
NH_FULL = 16
HD = 64


def build_program(T, D, HL, n_pat, blocks):
    """Build the per-core SPMD program.

    T: sequence length, D: model dim, HL: local heads, n_pat: number of
    distinct mixed-mask pattern tiles, blocks: per q-chunk list of
    (k_tile_index, pattern_index_or_None) for active score blocks.
    """
    CL = HL * HD            # local channels (q, k, or v width)
    NDT = D // 128          # d-tiles (contraction tiles for qkv matmuls)
    NTT = T // 128          # t-tiles
    QCW = min(512, T)       # q-chunk width
    NQC = T // QCW
    TPC = QCW // 128        # t-tiles per q-chunk
    NCT = CL // 128         # c-tiles for q/k/attn storage
    PCH = min(512, D)       # proj output chunk
    NPCH = D // PCH
    scale = 1.0 / math.sqrt(HD)

    nc = bacc.Bacc("TRN2", target_bir_lowering=False, debug=False)
    x = nc.dram_tensor("x", [T, D], F16, kind="ExternalInput").ap()
    wq = nc.dram_tensor("wq", [D, CL], F16, kind="ExternalInput").ap()
    wk = nc.dram_tensor("wk", [D, CL], F16, kind="ExternalInput").ap()
    wv = nc.dram_tensor("wv", [D, CL], F16, kind="ExternalInput").ap()
    bq = nc.dram_tensor("bq", [CL], F32, kind="ExternalInput").ap()
    bk = nc.dram_tensor("bk", [CL], F32, kind="ExternalInput").ap()
    bv = nc.dram_tensor("bv", [CL], F32, kind="ExternalInput").ap()
    wp = nc.dram_tensor("wp", [CL, D], F16, kind="ExternalInput").ap()
    bp = nc.dram_tensor("bp", [D], F32, kind="ExternalInput").ap()
    mp = nc.dram_tensor("mp", [max(n_pat, 1), 128, QCW], F16, kind="ExternalInput").ap()
    y = nc.dram_tensor("y", [T, D], F32, kind="ExternalOutput").ap()

    with tile.TileContext(nc) as tc, nc.allow_low_precision(
        reason="fp16 operands; matmul accumulates fp32 in PSUM"
    ):
        with ExitStack() as octx:
            persist = octx.enter_context(tc.tile_pool(name="persist", bufs=1))
            kT = [persist.tile([128, T], F16, name=f"kT{i}", tag=f"kT{i}") for i in range(NCT)]
            # Q^T natural layout [c, t]; scores matmuls contract K=64 (one
            # head's channels), with even/odd heads on partition halves
            # 0-63 / 64-127 -> disjoint PE row-groups run concurrently.
            qT = [persist.tile([128, T], F16, name=f"qT{i}", tag=f"qT{i}") for i in range(NCT)]
            # per-head stride 128 elements (256B) keeps the AV stationary
            # loads FWL-aligned; col HD of each slot is the ones column
            # that makes the AV matmul emit the softmax sums l on row HD.
            VSW = HL * 128
            vS = [persist.tile([128, VSW], F16, name=f"vS{i}", tag=f"vS{i}") for i in range(NTT)]
            for i in range(NTT):
                nc.gpsimd.memset(vS[i], 0.0)
                nc.gpsimd.memset(
                    vS[i].rearrange("p (h c) -> p h c", c=128)[:, :, HD:HD + 1], 1.0
                )
            attnT = [persist.tile([128, T], F16, name=f"attnT{i}", tag=f"attnT{i}") for i in range(NCT)]

            # resident weights: [128, NDT, CL] with layout (n p) c -> p n c,
            # so [:, dd, sl] is W[dd*128:(dd+1)*128, sl]. DMA issue order
            # below is ramp-critical: wvr first (the V matmuls start the
            # kernel), then chunk-0 x^T, then wq/wk.
            wvr = persist.tile([128, NDT, CL], F16, name="wvr", tag="wvr")
            wqr = persist.tile([128, NDT, CL], F16, name="wqr", tag="wqr")
            wkr = persist.tile([128, NDT, CL], F16, name="wkr", tag="wkr")
            bqs = persist.tile([128, NCT], F32, name="bqs", tag="bqs")
            bks = persist.tile([128, NCT], F32, name="bks", tag="bks")
            bvb = persist.tile([128, CL], F32, name="bvb", tag="bvb")
            wps = [persist.tile([128, D], F16, name=f"wps{i}", tag=f"wps{i}") for i in range(NCT)]
            bpb = persist.tile([128, D], F32, name="bpb", tag="bpb")
            mts = [persist.tile([128, QCW], F16, name=f"mt{i}", tag=f"mt{i}") for i in range(n_pat)]

            def load_weights_early():
                # V weights + the first x^T columns land first: the V
                # matmuls open the kernel. wq/wk interleave between the
                # remaining chunk-0 transposes so the QK matmuls aren't
                # starved right after the V part finishes.
                nc.sync.dma_start(out=wvr, in_=wv.rearrange("(n p) c -> p n c", p=128))

            def load_weights_mid():
                nc.sync.dma_start(out=wqr, in_=wq.rearrange("(n p) c -> p n c", p=128))

            def load_weights_late():
                nc.sync.dma_start(out=wkr, in_=wk.rearrange("(n p) c -> p n c", p=128))
                nc.sync.dma_start(
                    out=bvb,
                    in_=bass.AP(tensor=bv.tensor, offset=bv.offset, ap=[[0, 128]] + list(bv.ap)),
                )
                nc.sync.dma_start(out=bqs, in_=bq.rearrange("(m p) -> p m", p=128))
                nc.sync.dma_start(out=bks, in_=bk.rearrange("(m p) -> p m", p=128))

            def load_weights_last():
                for cc in range(NCT):
                    nc.sync.dma_start(out=wps[cc], in_=wp[cc * 128:(cc + 1) * 128, :])
                nc.sync.dma_start(
                    out=bpb,
                    in_=bass.AP(tensor=bp.tensor, offset=bp.offset, ap=[[0, 128]] + list(bp.ap)),
                )
                for i in range(n_pat):
                    nc.sync.dma_start(out=mts[i], in_=mp[i])

            xtp = octx.enter_context(tc.tile_pool(name="xtp", bufs=3))
            pab = octx.enter_context(tc.tile_pool(name="pab", bufs=2, space="PSUM"))
            pss = octx.enter_context(tc.tile_pool(name="pss", bufs=2, space="PSUM"))
            psav = octx.enter_context(tc.tile_pool(name="psav", bufs=2, space="PSUM"))
            ptl = octx.enter_context(tc.tile_pool(name="ptl", bufs=7))
            utp = octx.enter_context(tc.tile_pool(name="utp", bufs=5))
            ltp = octx.enter_context(tc.tile_pool(name="ltp", bufs=2))
            rvp = octx.enter_context(tc.tile_pool(name="rvp", bufs=2))
            rbp = octx.enter_context(tc.tile_pool(name="rbp", bufs=3))
            ysb = octx.enter_context(tc.tile_pool(name="ysb", bufs=2))
            drp = octx.enter_context(tc.tile_pool(name="drp", bufs=4, space="DRAM"))

            # ---- feeder: QKV compute for one chunk, as small PE items ----
            _xts = {}

            def issue_xT(ntc, mid_hook=None):
                if ntc in _xts or ntc >= NQC:
                    return _xts.get(ntc)
                xTc = xtp.tile([128, NDT, QCW], F16, name="xTc", tag="xTc")
                for dd in range(NDT):
                    nc.sync.dma_start_transpose(
                        xTc[:, dd, :],
                        x[ntc * QCW:(ntc + 1) * QCW, dd * 128:(dd + 1) * 128],
                    )
                    if dd == 1 and mid_hook is not None:
                        mid_hook()
                _xts[ntc] = xTc
                return xTc

            def abc_feeder(ntc):
                """Return a deque of zero-arg callables issuing chunk ntc's
                QKV matmuls in ~2-MM items. x^T transposes are prefetched
                two chunks deep."""
                xTc = issue_xT(ntc)
                issue_xT(ntc + 1)
                issue_xT(ntc + 2)
                tsl = slice(ntc * QCW, (ntc + 1) * QCW)
                items = deque()
                for tv in range(TPC):
                    tt = ntc * TPC + tv
                    box = {}
                    for dd0 in range(0, NDT, 2):
                        def v_item(dd0=dd0, box=box, tv=tv, tt=tt):
                            if dd0 == 0:
                                box["pv"] = pab.tile([128, CL], F32, name="pv", tag="pab")
                            for dd in (dd0, dd0 + 1):
                                nc.tensor.matmul(
                                    box["pv"],
                                    lhsT=xTc[:, dd, tv * 128:(tv + 1) * 128],
                                    rhs=wvr[:, dd, :],
                                    start=(dd == 0),
                                    stop=(dd == NDT - 1),
                                )
                            if dd0 == NDT - 2:
                                nc.vector.tensor_add(
                                    vS[tt].rearrange("p (h c) -> p h c", c=128)[:, :, 0:HD],
                                    box["pv"].rearrange("p (h d) -> p h d", h=HL),
                                    bvb.rearrange("p (h d) -> p h d", h=HL),
                                )
                        items.append(v_item)
                for mi in range(2 * NCT):
                    isq = mi < NCT
                    mc = mi % NCT
                    wsrc = wqr if isq else wkr
                    box = {}
                    for dd0 in range(0, NDT, 2):
                        def qk_item(dd0=dd0, box=box, mc=mc, isq=isq, wsrc=wsrc):
                            if dd0 == 0:
                                box["pb"] = pab.tile([128, QCW], F32, name="pb", tag="pab")
                            for dd in (dd0, dd0 + 1):
                                nc.tensor.matmul(
                                    box["pb"],
                                    lhsT=wsrc[:, dd, mc * 128:(mc + 1) * 128],
                                    rhs=xTc[:, dd, :],
                                    start=(dd == 0),
                                    stop=(dd == NDT - 1),
                                )
                            if dd0 == NDT - 2:
                                pb = box["pb"]
                                if isq:
                                    nc.vector.tensor_scalar_add(
                                        qT[mc][:, tsl], pb, bqs[:, mc:mc + 1]
                                    )
                                else:
                                    nc.vector.tensor_scalar_add(
                                        kT[mc][:, tsl], pb, bks[:, mc:mc + 1]
                                    )
                        items.append(qk_item)
                return items

            def proj_feeder(qc):
                """Output projection for q-chunk qc (reads normalized attnT)."""
                items = deque()
                for tv in range(TPC):
                    tt = qc * TPC + tv
                    box = {}
                    for nch in range(NPCH):
                        for cc0 in range(0, NCT, 2):
                            def p_item(cc0=cc0, nch=nch, box=box, tt=tt):
                                if nch == 0 and cc0 == 0:
                                    box["yt"] = ysb.tile([128, D], F32, name="yt", tag="yt")
                                if cc0 == 0:
                                    box["py"] = pab.tile([128, PCH], F32, name="py", tag="pab")
                                for cc in (cc0, cc0 + 1):
                                    nc.tensor.matmul(
                                        box["py"],
                                        lhsT=attnT[cc][:, tt * 128:(tt + 1) * 128],
                                        rhs=wps[cc][:, nch * PCH:(nch + 1) * PCH],
                                        start=(cc == 0),
                                        stop=(cc == NCT - 1),
                                    )
                                if cc0 == NCT - 2:
                                    nc.vector.tensor_add(
                                        box["yt"][:, nch * PCH:(nch + 1) * PCH],
                                        box["py"],
                                        bpb[:, nch * PCH:(nch + 1) * PCH],
                                    )
                                    if nch == NPCH - 1:
                                        nc.sync.dma_start(
                                            out=y[tt * 128:(tt + 1) * 128, :], in_=box["yt"]
                                        )
                            items.append(p_item)
                return items

            abc_q = deque()
            proj_q = deque()

            def pump(n):
                for _ in range(n):
                    if abc_q:
                        abc_q.popleft()()
                    elif proj_q:
                        proj_q.popleft()()
                    else:
                        return

            def drain_abc():
                while abc_q:
                    abc_q.popleft()()

            # ---- flash attention for one q-chunk ----
            # Heads run in even/odd pairs: the two K=64 score matmuls live
            # on disjoint PE row-halves (partitions 0-63 / 64-127) and
            # execute concurrently in the systolic array.
            def flash(qc):
                # masked (diagonal) k-tiles first: their GpSimd select chain
                # then overlaps the clean full blocks instead of serializing
                # into the next head-pair / the projection tail.
                row = list(reversed(blocks[qc]))
                assert row, f"q-chunk {qc} has no active k-tiles"
                qsl = slice(qc * QCW, (qc + 1) * QCW)
                groups = [row[i:i + 2] for i in range(0, len(row), 2)]
                for hp in range(NCT):
                    pavs = [
                        psav.tile([128, QCW], F32, name="pav", tag="pav")
                        for _ in range(2)
                    ]
                    # AV issue runs two k-tile groups behind the scores:
                    # the first AV (which waits on the previous head-pair's
                    # PSUM evacuation) then has extra slack, and every AV's
                    # exp has long since finished.
                    depth = min(2, len(groups) - 1) or 1
                    pending = []
                    bi = 0
                    for gi, grp in enumerate(groups):
                        # both parities' S^T blocks for a k-tile share one pS
                        # tile (cols 0:512 even head, 512:1024 odd head): one
                        # exp serves both, and the two K=64 matmuls issue
                        # back-to-back so they run concurrently on disjoint
                        # PE row-halves. Two consecutive k-tiles' S quads
                        # also issue together so the second pair rides the
                        # first pair's fill instead of paying its drain.
                        pSs = []
                        for ki, _ in grp:
                            pS = pss.tile([128, 2 * QCW], F32, name="pS", tag="pS")
                            for par in range(2):
                                rsl = slice(par * HD, (par + 1) * HD)
                                nc.tensor.matmul(
                                    pS[:, par * QCW:(par + 1) * QCW],
                                    lhsT=kT[hp][rsl, ki * 128:(ki + 1) * 128],
                                    rhs=qT[hp][rsl, qsl],
                                    start=True,
                                    stop=True,
                                )
                            pSs.append(pS)
                        ents = []
                        for (ki, pat), pS in zip(grp, pSs):
                            pT = ptl.tile([128, 2 * QCW], F16, name="pT", tag="pT")
                            nc.scalar.activation(pT, pS, AF.Exp, scale=scale)
                            if pat is not None:
                                kind, arg, pidx = pat
                                for par in range(2):
                                    sl = pT[:, par * QCW:(par + 1) * QCW]
                                    if kind == "tri":
                                        # keep where (q - k) >= 0, else 0
                                        nc.gpsimd.affine_select(
                                            out=sl,
                                            in_=sl,
                                            pattern=[[1, QCW]],
                                            base=arg,
                                            channel_multiplier=-1,
                                            compare_op=mybir.AluOpType.is_ge,
                                            fill=0.0,
                                        )
                                    else:
                                        nc.gpsimd.tensor_mul(sl, sl, mts[pidx])
                            ents.append((pT, ki, bi))
                            bi += 1
                        pump(1)
                        pending.append(ents)
                        if len(pending) > depth:
                            issue_av(hp, pavs, pending.pop(0), False)
                        pump(1)
                    while pending:
                        issue_av(hp, pavs, pending.pop(0), not pending)
                    # evacuate unnormalized attn rows + l row; frees banks.
                    # In the first (short) chunk the vector queue is jammed
                    # with the pumped QKV bias-evacuations while the scalar
                    # engine has slack, so evacuate there instead.
                    uts = []
                    for par in range(2):
                        ut = utp.tile([HD + 1, QCW], F16, name="ut", tag="ut")
                        if qc == 0:
                            nc.scalar.copy(ut, pavs[par][0:HD + 1, :])
                        else:
                            nc.vector.tensor_copy(ut, pavs[par][0:HD + 1, :])
                        uts.append(ut)
                    hp_norm(qc, hp, uts)
                    pump(2)

            def issue_av(hp, pavs, ents, is_last):
                for ei, (pT, ki, bi) in enumerate(ents):
                    for par in range(2):
                        h = 2 * hp + par
                        nc.tensor.matmul(
                            pavs[par],
                            lhsT=vS[ki][:, h * 128:h * 128 + 128],
                            rhs=pT[:, par * QCW:(par + 1) * QCW],
                            start=(bi == 0),
                            stop=(is_last and ei == len(ents) - 1),
                        )

            def hp_norm(qc, hp, uts):
                """1/l for one head pair. The two l rows bounce through DRAM
                so the reciprocal can run on a [64, 16] reshape (reciprocal
                cost is free-dim-bound: 16 elems/lane, not 512), then
                broadcast back as [64, QCW] and multiply into attnT."""
                lb = drp.tile([2, QCW], F16, name="lb", tag="lb")
                for par in range(2):
                    nc.sync.dma_start(out=lb[par:par + 1, :], in_=uts[par][HD:HD + 1, :])
                nfd = 2 * QCW // 64
                lsb = ltp.tile([64, nfd], F16, name="lsb", tag="lsb")
                nc.sync.dma_start(
                    out=lsb,
                    in_=bass.AP(tensor=lb.tensor, offset=lb.offset, ap=[[nfd, 64], [1, nfd]]),
                )
                rinv = rvp.tile([64, nfd], F16, name="rinv", tag="rinv")
                nc.vector.reciprocal(rinv, lsb)
                scr = drp.tile([2, QCW], F16, name="scr", tag="scr")
                nc.sync.dma_start(
                    out=bass.AP(tensor=scr.tensor, offset=scr.offset, ap=[[nfd, 64], [1, nfd]]),
                    in_=rinv,
                )
                for par in range(2):
                    row = scr[par:par + 1, :]
                    rbs = rbp.tile([HD, QCW], F16, name="rbs", tag="rbs")
                    nc.sync.dma_start(
                        out=rbs,
                        in_=bass.AP(tensor=row.tensor, offset=row.offset, ap=[[0, HD]] + list(row.ap)[1:]),
                    )
                    nc.vector.tensor_mul(
                        attnT[hp][par * HD:(par + 1) * HD, qc * QCW:(qc + 1) * QCW],
                        uts[par][0:HD, :],
                        rbs,
                    )

            # ---- fused main loop ----
            load_weights_early()
            issue_xT(0, mid_hook=load_weights_mid)
            load_weights_late()
            abc_q.extend(abc_feeder(0))
            load_weights_last()
            drain_abc()
            for ntc in range(NQC):
                if ntc + 1 < NQC:
                    abc_q.extend(abc_feeder(ntc + 1))
                flash(ntc)
                drain_abc()
                proj_q.extend(proj_feeder(ntc))
            while proj_q:
                proj_q.popleft()()
    nc.compile()
    return nc


def classify_mask(mask_bool, T):
    """Classify S^T blocks [k-tile 128, q-chunk 512] as skip / full / mixed.

    mask_bool: [T, T] bool, mask_bool[q, k] = attend(q -> k).
    Returns (blocks, patterns): blocks[qc] = list of (ki, pat_idx|None),
    patterns = np.ndarray [n_pat, 128, QCW] float32.
    """
    QCW = min(512, T)
    NQC = T // QCW
    NKT = T // 128
    maskT = mask_bool.T  # [k, q]
    patterns = []
    pat_index = {}
    blocks = []

    def register(blk):
        key = blk.tobytes()
        if key not in pat_index:
            pat_index[key] = len(patterns)
            patterns.append(blk.astype(np.float32))
        return pat_index[key]

    for qc in range(NQC):
        row = []
        for ki in range(NKT):
            blk = maskT[ki * 128:(ki + 1) * 128, qc * QCW:(qc + 1) * QCW]
            if not blk.any():
                continue
            if blk.all():
                row.append((ki, None))
                continue
            # every mixed block also gets a multiplicative pattern tile so
            # the kernel can mask one parity on the vector engine
            pidx = register(blk)
            # tril-offset block? keep iff k <= q, i.e. p <= base + f
            base = qc * QCW - ki * 128
            p = np.arange(128)[:, None]
            f = np.arange(QCW)[None, :]
            if np.array_equal(blk, p <= base + f):
                row.append((ki, ("tri", base, pidx)))
                continue
            row.append((ki, ("pat", None, pidx)))
        blocks.append(row)
    n_pat = len(patterns)
    if patterns:
        pats = np.stack(patterns)
    else:
        pats = np.zeros((1, 128, QCW), np.float32)
    return blocks, pats, n_pat


_prog_cache = {}


def _get_program(T, D, HL, mask_bool):
    key = (T, D, HL, mask_bool.tobytes())
    if key not in _prog_cache:
        blocks, pats, n_pat = classify_mask(mask_bool, T)
        nc = build_program(T, D, HL, n_pat, blocks)
        _prog_cache[key] = (nc, blocks, pats)
    return _prog_cache[key]


def kernel(x, W_qkv, b_qkv, W_proj, b_proj, mask):
    out, _ = run_attention(x, W_qkv, b_qkv, W_proj, b_proj, mask)
    return out


def run_attention(x, W_qkv, b_qkv, W_proj, b_proj, mask, trace=False):
    x = np.ascontiguousarray(np.asarray(x, dtype=np.float32))
    W_qkv = np.asarray(W_qkv, dtype=np.float32)
    b_qkv = np.asarray(b_qkv, dtype=np.float32)
    W_proj = np.asarray(W_proj, dtype=np.float32)
    b_proj = np.asarray(b_proj, dtype=np.float32)
    Bc, T, D = x.shape
    NH = NH_FULL
    HL = NH // 2  # heads per core (two head-groups)
    CL = HL * HD

    mask_bool = np.asarray(mask)[0, 0] != 0

    nc, blocks, pats = _get_program(T, D, HL, mask_bool)

    in_maps = []
    n_cores = 2 * Bc
    for c in range(n_cores):
        b, g = c // 2, c % 2
        sl = slice(g * CL, (g + 1) * CL)
        in_maps.append({
            "x": np.ascontiguousarray(x[b]).astype(np.float16),
            "wq": np.ascontiguousarray(W_qkv[:, 0 * D:1 * D][:, sl]).astype(np.float16),
            "wk": np.ascontiguousarray(W_qkv[:, 1 * D:2 * D][:, sl]).astype(np.float16),
            "wv": np.ascontiguousarray(W_qkv[:, 2 * D:3 * D][:, sl]).astype(np.float16),
            "bq": np.ascontiguousarray(b_qkv[0 * D:1 * D][sl]),
            "bk": np.ascontiguousarray(b_qkv[1 * D:2 * D][sl]),
            "bv": np.ascontiguousarray(b_qkv[2 * D:3 * D][sl]),
            "wp": np.ascontiguousarray(W_proj[sl, :]).astype(np.float16),
            "bp": b_proj if g == 0 else np.zeros_like(b_proj),
            "mp": pats.astype(np.float16),
        })

    res = run_bass_kernel_spmd(nc, in_maps, list(range(n_cores)), trace=trace)
    out = np.empty((Bc, T, D), np.float32)
    for b in range(Bc):
        out[b] = res.results[2 * b]["y"] + res.results[2 * b + 1]["y"]
    return out, res
